# revision 41
# baseline (speedup 1.0000x reference)
"""Trainium2 Bass kernel for fused attention layer (QKV proj + QKNorm + RoPE +
causal attention + out proj), tensor-parallel across (batch, head-group) on 8
NeuronCores.

Reference semantics (B=2, L=2048, D=2048, H=16, HD=128):
    qkv = x @ w_qkv + b_qkv ; split q,k,v per head
    q,k = rms_norm(q)*q_scale, rms_norm(k)*k_scale   (over head_dim)
    q,k = rope(q), rope(k)                           (rotate-half)
    attn = softmax(mask(q k^T / sqrt(HD)))
    out = (attn @ v) reshaped @ w_out + b_out

Sharding: core c -> batch b=c//4, heads 4g..4g+3 with g=c%4. Each core emits a
partial out-projection [L, D]; the host sums the 4 partials per batch + b_out.
"""

import numpy as np

import concourse.bass as bass
import concourse.bacc as bacc
import concourse.tile as tile
import concourse.mybir as mybir
from concourse.bass_utils import run_bass_kernel_spmd

f32 = mybir.dt.float32
f16 = mybir.dt.float16
f32r = mybir.dt.float32r
bf16 = mybir.dt.bfloat16
AF = mybir.ActivationFunctionType
ALU = mybir.AluOpType

B = 2
D = 2048
H = 16
HD = 128
KC = D // 128          # 16 contraction chunks of 128
EPS = 1e-6
ROPE_THETA = 10000.0
NEG = -1e30
N_CORES = 8
HEADS_PER_CORE = 4     # 2 pairs of 2


def build_nc(L):
    TQ = L // 512       # 512-wide t chunks (query chunks, proj chunks)
    TK = L // 128       # 128-wide t chunks (key chunks, v chunks, out-proj chunks)
    XC = L // 256       # 256-wide x-stream chunks

    nc = bacc.Bacc(None, target_bir_lowering=False)

    xT_d = nc.dram_tensor("xT", [128, KC, L], f32r, kind="ExternalInput")
    wqk_d = nc.dram_tensor("wqk", [2, 128, KC, 512], f32r, kind="ExternalInput")
    wv_d = nc.dram_tensor("wv", [2, 128, KC, 256], f32r, kind="ExternalInput")
    wout_d = nc.dram_tensor("wout", [128, 4, D], f32r, kind="ExternalInput")
    cos_d = nc.dram_tensor("cosT", [128, L], f16, kind="ExternalInput")
    sin_d = nc.dram_tensor("sinT", [128, L], f16, kind="ExternalInput")  # [-sin; +sin]
    bqk_d = nc.dram_tensor("b_qk", [128, 8], f32, kind="ExternalInput")
    qsks_d = nc.dram_tensor("qs_ks", [128, 2], f32, kind="ExternalInput")
    mask_d = nc.dram_tensor("maskneg", [128, 896], bf16, kind="ExternalInput")
    ones_d = nc.dram_tensor("ones", [128, 1], f32r, kind="ExternalInput")
    onesb_d = nc.dram_tensor("ones_bf", [128, 1], bf16, kind="ExternalInput")
    out_d = nc.dram_tensor("out_p", [L, D], f32, kind="ExternalOutput")

    inv_sqrt_hd = float(1.0 / np.sqrt(HD))

    # register EPS as a const AP (activation float biases need one)
    _eps_t = nc.alloc_sbuf_tensor(f"const-float32-{EPS}", [128, 1], f32)
    nc.gpsimd.memset(_eps_t.ap(), EPS)
    nc.const_aps.aps[(f32, EPS)] = _eps_t.ap()
    nc.all_engine_barrier()

    from contextlib import ExitStack

    with ExitStack() as ctx:
        tc = ctx.enter_context(tile.TileContext(nc))
        ctx.enter_context(
            nc.allow_low_precision(
                reason="f32r rounding of matmul operands is intentional"
            )
        )
        px = ctx.enter_context(tc.tile_pool(name="px", bufs=11))      # x stream
        pw = ctx.enter_context(tc.tile_pool(name="pw", bufs=1))       # wqk + wv
        pqk = ctx.enter_context(tc.tile_pool(name="pqk", bufs=4))     # qT/kT
        pv = ctx.enter_context(tc.tile_pool(name="pv", bufs=1))       # v slab
        pat = ctx.enter_context(tc.tile_pool(name="pat", bufs=4))     # attn_outT
        ptab = ctx.enter_context(tc.tile_pool(name="ptab", bufs=1))   # constants
        pscr = ctx.enter_context(tc.tile_pool(name="pscr", bufs=1))   # scratch
        pexp = ctx.enter_context(tc.tile_pool(name="pexp", bufs=2))   # exp tiles
        pbc = ctx.enter_context(tc.tile_pool(name="pbc", bufs=2))     # broadcasts
        psmall = ctx.enter_context(tc.tile_pool(name="psmall", bufs=2))
        ppsum = ctx.enter_context(tc.tile_pool(name="psum", bufs=3, space="PSUM"))
        ppsum_sm = ctx.enter_context(tc.tile_pool(name="psum_sm", bufs=2, space="PSUM"))
        if True:
            # ---- resident constants ----
            cosT = ptab.tile([128, L], f16, tag="cos")
            sinT = ptab.tile([128, L], f16, tag="sin")
            bqk = ptab.tile([128, 8], f32, tag="bqk")
            qsks = ptab.tile([128, 2], f32, tag="qsks")
            masks = ptab.tile([128, 896], bf16, tag="masks")
            ones = ptab.tile([128, 1], f32r, tag="ones")
            onesb = ptab.tile([128, 1], bf16, tag="onesb")
            nc.sync.dma_start(cosT[:], cos_d[:])
            nc.sync.dma_start(sinT[:], sin_d[:])
            nc.sync.dma_start(bqk[:], bqk_d[:])
            nc.sync.dma_start(qsks[:], qsks_d[:])
            nc.sync.dma_start(masks[:], mask_d[:])
            nc.sync.dma_start(ones[:], ones_d[:])
            nc.sync.dma_start(onesb[:], onesb_d[:])

            attnT = {}  # (pair, hh) -> [128, L] f32r

            def gen_attn(pair, hh, j, qk, vt, aT):
                """Generator emitting one (head, q-chunk) attention unit,
                yielding after each 2-block group so projection work can be
                interleaved between groups."""
                qT = qk[(0, hh)]
                kT = qk[(1, hh)]
                jsl = slice(j * 512, j * 512 + 512)
                ncc = 4 * (j + 1)
                ps_o_t = ppsum.tile([128, 1024], f32, tag="big", name="ps_o")
                ps_o = ps_o_t[:, 0:512]
                ps_sum = ps_o_t[0:1, 512:1024]
                for cp in range(0, ncc, 2):
                    ps_s = ppsum.tile([128, 1024], f32, tag="big", name="ps_s")
                    for ci in range(2):
                        c = cp + ci
                        nc.tensor.matmul(
                            ps_s[:, ci * 512 : ci * 512 + 512],
                            kT[:, c * 128 : c * 128 + 128],
                            qT[:, jsl],
                            start=True,
                            stop=True,
                        )
                        r = c - 4 * j
                        if r >= 0:
                            ms0 = 384 - 128 * r
                            nc.vector.tensor_tensor(
                                ps_s[:, ci * 512 : ci * 512 + 512],
                                ps_s[:, ci * 512 : ci * 512 + 512],
                                masks[:, ms0 : ms0 + 512],
                                ALU.add,
                            )
                    e = pexp.tile([128, 1024], f32r, tag="e")
                    nc.scalar.activation(e[:], ps_s[:], AF.Exp, scale=inv_sqrt_hd)
                    for ci in range(2):
                        c = cp + ci
                        esl = e[:, ci * 512 : ci * 512 + 512]
                        nc.tensor.matmul(
                            ps_sum, ones[:], esl,
                            start=(c == 0), stop=(c == ncc - 1),
                        )
                        nc.tensor.matmul(
                            ps_o, vt[hh][:, c, :], esl,
                            start=(c == 0), stop=(c == ncc - 1),
                        )
                    yield
                # raw copy + reciprocal free the psum; normalize trails
                nc.vector.tensor_copy(aT[:, jsl], ps_o)
                reca = psmall.tile([1, 512], f32, tag="small")
                nc.vector.reciprocal_approx_fast(reca[:], ps_sum)
                rbc = pbc.tile([128, 512], f32, tag="bc")
                nc.gpsimd.partition_broadcast(rbc[:], reca[:])
                nc.vector.tensor_tensor(
                    aT[:, jsl], aT[:, jsl].bitcast(f32), rbc[:], ALU.mult
                )

            from collections import deque

            gens = deque()

            def advance(n):
                for _ in range(n):
                    while gens:
                        try:
                            next(gens[0][1])
                            break
                        except StopIteration:
                            gens.popleft()
                    if not gens:
                        break

            def drain():
                while gens:
                    try:
                        next(gens[0][1])
                    except StopIteration:
                        gens.popleft()

            def drain_until(j_max):
                # fully drain every pending generator labelled <= j_max
                # (FIFO front is always oldest; labels are per-pair ascending)
                while any(lbl <= j_max for lbl, _ in gens):
                    try:
                        next(gens[0][1])
                    except StopIteration:
                        gens.popleft()

            for pair in range(2):
                # ---- weights for this pair ----
                wqk = pw.tile([128, KC, 512], f32r, tag="wqk")
                wv = pw.tile([128, KC, 256], f32r, tag="wv")
                for kq in range(4):
                    ksl = slice(kq * (KC // 4), (kq + 1) * (KC // 4))
                    nc.sync.dma_start(wqk[:, ksl], wqk_d[pair][:, ksl])
                    nc.sync.dma_start(wv[:, ksl], wv_d[pair][:, ksl])

                # per-head v slabs so pair p+1's v-proj only waits on the
                # matching head's attention
                vt = [
                    pv.tile([128, TK, 128], f32r, tag=f"v{hh}", name=f"v_{hh}")
                    for hh in range(2)
                ]
                qk = {}
                for qki in range(2):
                    for hh in range(2):
                        qk[(qki, hh)] = pqk.tile(
                            [128, L], f32r, tag="qk", name=f"qk_{qki}_{hh}"
                        )
                aTs = {}
                for hh in range(2):
                    aTs[hh] = pat.tile([128, L], f32r, tag="attnT", name=f"aT_{hh}")
                    attnT[(pair, hh)] = aTs[hh]

                # ---- projection + norm + rope + interleaved attention ----
                for c4 in range(TQ):
                    t0 = c4 * 512
                    KQ = KC // 8
                    xts = []
                    for kh in range(8):
                        xt = px.tile([128, KQ, 512], f32r, tag="x", name=f"xt_{kh}")
                        nc.sync.dma_start(
                            xt[:], xT_d[:, kh * KQ : (kh + 1) * KQ, t0 : t0 + 512]
                        )
                        xts.append(xt)
                    # fill the x-DMA window with pending attention blocks
                    advance(10)

                    def norm_stageA(ps, bidx, qki):
                        b_ap = bqk[:, bidx : bidx + 1]
                        sq = pscr.tile([128, 512], bf16, tag="sq")
                        nc.scalar.activation(sq[:], ps[:], AF.Square, bias=b_ap)
                        q_sb = pscr.tile([128, 512], f32, tag="q_sb")
                        nc.vector.tensor_scalar(
                            q_sb[:], ps[:], b_ap, qsks[:, qki : qki + 1],
                            ALU.add, ALU.mult,
                        )
                        rot = pscr.tile([128, 512], f32, tag="rot")
                        nc.sync.dma_start(rot[0:64, :], q_sb[64:128, :])
                        nc.sync.dma_start(rot[64:128, :], q_sb[0:64, :])
                        return sq, q_sb, rot

                    def norm_stageB(sq, q_sb, rot, qki, hh, tsl):
                        # deferred: the ms matmul runs once ACT's square is
                        # long done, so PE never stalls on it
                        ms = ppsum_sm.tile([1, 512], f32, tag="sm", name="ms")
                        nc.tensor.matmul(
                            ms[:], onesb[:], sq[:], start=True, stop=True
                        )
                        sms = psmall.tile([1, 512], f32, tag="small")
                        nc.scalar.activation(
                            sms[:], ms[:], AF.Sqrt, bias=EPS, scale=float(1.0 / HD)
                        )
                        rec = psmall.tile([1, 512], f32, tag="small")
                        nc.vector.reciprocal_approx_fast(rec[:], sms[:])
                        rstd = pbc.tile([128, 512], f32, tag="bc")
                        nc.gpsimd.partition_broadcast(rstd[:], rec[:])
                        t1 = pscr.tile([128, 512], f32, tag="sq", name="t1")
                        nc.vector.tensor_tensor(t1[:], q_sb[:], cosT[:, tsl], ALU.mult)
                        nc.vector.tensor_tensor(rot[:], rot[:], sinT[:, tsl], ALU.mult)
                        nc.vector.tensor_tensor(t1[:], t1[:], rot[:], ALU.add)
                        nc.vector.tensor_tensor(
                            qk[(qki, hh)][:, tsl], t1[:], rstd[:], ALU.mult
                        )

                    pending_B = None

                    # v projection first (attention j=c4 needs v chunks <= c4)
                    for tsub in range(4):
                        ps_v = ppsum_sm.tile([128, 256], f32, tag="sm", name="ps_v")
                        for kc in range(KC):
                            nc.tensor.matmul(
                                ps_v[:],
                                xts[kc // KQ][
                                    :, kc % KQ, tsub * 128 : tsub * 128 + 128
                                ],
                                wv[:, kc, :],
                                start=(kc == 0),
                                stop=(kc == KC - 1),
                            )
                        vi = c4 * 4 + tsub
                        for hh in range(2):
                            nc.vector.tensor_copy(
                                vt[hh][:, vi, :],
                                ps_v[:, hh * 128 : hh * 128 + 128],
                            )
                        advance(2)

                    # q/k projections + split norm/rope, one instance at a time
                    for qki in range(2):
                        for hh in range(2):
                            ps = ppsum_sm.tile(
                                [128, 512], f32, tag="sm", name=f"psqk_{qki}_{hh}"
                            )
                            col = (qki * 2 + hh) * 128
                            for kc in range(KC):
                                nc.tensor.matmul(
                                    ps[:],
                                    wqk[:, kc, col : col + 128],
                                    xts[kc // KQ][:, kc % KQ, :],
                                    start=(kc == 0),
                                    stop=(kc == KC - 1),
                                )
                            if pending_B is not None:
                                pending_B()
                            bidx = pair * 4 + qki * 2 + hh
                            tsl = slice(c4 * 512, c4 * 512 + 512)
                            sq, q_sb, rot = norm_stageA(ps, bidx, qki)
                            pending_B = (
                                lambda sq=sq, q_sb=q_sb, rot=rot, qki=qki,
                                hh=hh, tsl=tsl: norm_stageB(
                                    sq, q_sb, rot, qki, hh, tsl
                                )
                            )
                            advance(2)
                    pending_B()
                    pending_B = None

                    for hh in range(2):
                        gens.append((c4, gen_attn(pair, hh, c4, qk, vt, aTs[hh])))
            # ---- out projection, interleaved with the attention drain ----
            wo = pw.tile([128, 4, D], f32r, tag="wqk", name="wo")
            nc.sync.dma_start(wo[:], wout_d[:])
            for tc_i in range(TK):
                # attnT writes for this t-chunk must be EMITTED before the
                # reads (Tile deps follow emission order)
                drain_until(tc_i // 4)
                tsl = slice(tc_i * 128, tc_i * 128 + 128)
                for n2 in range(D // 1024):
                    ps = ppsum.tile([128, 1024], f32, tag="big", name="ps_out")
                    for half in range(2):
                        nsl = slice(n2 * 1024 + half * 512, n2 * 1024 + half * 512 + 512)
                        for hi in range(4):
                            nc.tensor.matmul(
                                ps[:, half * 512 : half * 512 + 512],
                                attnT[(hi // 2, hi % 2)][:, tsl],
                                wo[:, hi, nsl],
                                start=(hi == 0),
                                stop=(hi == 3),
                            )
                    o = pexp.tile([128, 1024], f32, tag="e", name="o_stage")
                    nc.vector.tensor_copy(o[:], ps[:])
                    nc.sync.dma_start(
                        out_d[tsl, n2 * 1024 : n2 * 1024 + 1024], o[:]
                    )
                    advance(2)
            drain()

    nc.compile()
    return nc


def host_inputs(x, w_qkv, b_qkv, q_scale, k_scale, w_out, L):
    """Build the 8 per-core input maps."""
    x = np.asarray(x, np.float32)
    w_qkv = np.asarray(w_qkv, np.float32)
    b_qkv = np.asarray(b_qkv, np.float32)
    w_out = np.asarray(w_out, np.float32)
    q_scale = np.asarray(q_scale, np.float32)
    k_scale = np.asarray(k_scale, np.float32)

    half = HD // 2
    inv_freq = 1.0 / (ROPE_THETA ** (np.arange(half, dtype=np.float64) / half))
    pos = np.arange(L, dtype=np.float64)
    ang = pos[None, :] * inv_freq[:, None]          # [64, L]
    cos_t = np.cos(ang)
    sin_t = np.sin(ang)
    import ml_dtypes
    cosT = np.concatenate([cos_t, cos_t], 0).astype(np.float16)   # [128, L]
    sinT = np.concatenate([-sin_t, sin_t], 0).astype(np.float16)  # [-s; +s]

    # consolidated straddle mask: M[i, u] = 0 iff u >= i + 384 else NEG;
    # slice [384-128r : 896-128r] gives the r-straddle [128, 512] mask
    ii = np.arange(128)[:, None]
    uu = np.arange(896)[None, :]
    maskneg = np.ascontiguousarray(
        np.where(uu >= ii + 384, 0.0, NEG).astype(ml_dtypes.bfloat16)
    )

    onesv = np.ones((128, 1), np.float32)
    onesb = np.ones((128, 1), ml_dtypes.bfloat16)
    qsks = np.stack([q_scale, k_scale], 1)          # [128, 2]

    in_maps = []
    for c in range(N_CORES):
        b = c // 4
        g = c % 4
        heads = [4 * g + i for i in range(4)]
        xT = np.ascontiguousarray(x[b].T)                       # [D, L]
        xTr = np.ascontiguousarray(
            xT.reshape(KC, 128, L).transpose(1, 0, 2)
        )                                                        # [128, KC, L]
        wqk = np.empty((2, 128, KC, 512), np.float32)
        wv = np.empty((2, 128, KC, 256), np.float32)
        b_qk = np.empty((128, 2, 2, 2), np.float32)  # reshaped to [128, 8] below
        for p in range(2):
            hp = heads[2 * p : 2 * p + 2]
            cols = np.concatenate(
                [
                    np.arange(qki * D + h * HD, qki * D + (h + 1) * HD)
                    for qki in range(2)
                    for h in hp
                ]
            )
            wqk[p] = w_qkv[:, cols].reshape(KC, 128, 512).transpose(1, 0, 2)
            vcols = np.concatenate(
                [np.arange(2 * D + h * HD, 2 * D + (h + 1) * HD) for h in hp]
            )
            wv[p] = w_qkv[:, vcols].reshape(KC, 128, 256).transpose(1, 0, 2)
            for qki in range(2):
                for hh in range(2):
                    b_qk[:, p, qki, hh] = b_qkv[
                        qki * D + hp[hh] * HD : qki * D + (hp[hh] + 1) * HD
                    ]

        wout = (
            w_out[heads[0] * HD : (heads[-1] + 1) * HD]
            .reshape(4, 128, D)
            .transpose(1, 0, 2)
        )
        in_maps.append(
            {
                "xT": np.ascontiguousarray(xTr),
                "wqk": np.ascontiguousarray(wqk),
                "wv": np.ascontiguousarray(wv),
                "wout": np.ascontiguousarray(wout),
                "cosT": cosT,
                "sinT": sinT,
                "b_qk": np.ascontiguousarray(b_qk.reshape(128, 8)),
                "qs_ks": np.ascontiguousarray(qsks),
                "maskneg": maskneg,
                "ones": onesv,
                "ones_bf": onesb,
            }
        )
    return in_maps


_NC_CACHE = {}


def _get_nc(L):
    if L not in _NC_CACHE:
        _NC_CACHE[L] = build_nc(L)
    return _NC_CACHE[L]


def run(x, w_qkv, b_qkv, q_scale, k_scale, w_out, b_out, L, **rb_kwargs):
    nc = _get_nc(L)
    in_maps = host_inputs(x, w_qkv, b_qkv, q_scale, k_scale, w_out, L)
    res = run_bass_kernel_spmd(nc, in_maps, list(range(N_CORES)), **rb_kwargs)
    parts = np.stack([r["out_p"] for r in res.results])          # [8, L, D]
    b_v = np.asarray(b_qkv, np.float64)[2 * D : 3 * D]
    bias_eff = np.asarray(b_out, np.float64) + b_v @ np.asarray(w_out, np.float64)
    out = np.empty((B, L, D), np.float32)
    for b in range(B):
        out[b] = parts[4 * b : 4 * b + 4].sum(0, dtype=np.float64) + bias_eff
    return out, res


def kernel(x, w_qkv, b_qkv, q_scale, k_scale, w_out, b_out, mask):
    out, _ = run(x, w_qkv, b_qkv, q_scale, k_scale, w_out, b_out, L=x.shape[1])
    return out
